# revision 1
# baseline (speedup 1.0000x reference)
"""Trainium2 Bass kernel for nn_ComposedFeatureTransformer (embedding lookup).

out_s[b, :] = bias + sum_k values_s[b, k] * merged_weight[indices_s[b, k], :]
for s in {0, 1}.

Strategy: data-parallel over the batch dim across 8 NeuronCores (512 rows
per core, both feature sets on every core). The 186 MB weight table stays
in each core's HBM; rows are fetched with indirect (gather) DMA, 128 rows
per instruction. ACT applies the per-(batch,k) value scale, DVE
accumulates, bias is folded into the k=0 accumulation.
"""

import numpy as np

import concourse.bacc as bacc
import concourse.bass as bass
import concourse.mybir as mybir
import concourse.tile as tile
from concourse.bass_utils import run_bass_kernel_spmd

N_CORES = 8
BATCH = 4096
PER_CORE = BATCH // N_CORES  # 512
K = 32
V = 45056
D = 1032
P = 128
N_TILES = PER_CORE // P  # 4

TRACE = False  # set by test harness to collect an NTFF profile
LAST_RESULT = None  # BassKernelResults of the last run (for profiling)

_NC = None


def _build():
    nc = bacc.Bacc("TRN2", debug=False, num_devices=N_CORES)
    f32 = mybir.dt.float32
    i32 = mybir.dt.int32

    idx_d = [
        nc.dram_tensor(f"idx{s}", [PER_CORE, K], i32, kind="ExternalInput")
        for s in range(2)
    ]
    val_d = [
        nc.dram_tensor(f"val{s}", [PER_CORE, K], f32, kind="ExternalInput")
        for s in range(2)
    ]
    w_d = nc.dram_tensor("weight", [V, D], f32, kind="ExternalInput")
    bias_d = nc.dram_tensor("bias_rep", [P, D], f32, kind="ExternalInput")
    out_d = [
        nc.dram_tensor(f"out{s}", [PER_CORE, D], f32, kind="ExternalOutput")
        for s in range(2)
    ]

    with tile.TileContext(nc) as tc:
        with (
            tc.tile_pool(name="const", bufs=1) as const_pool,
            tc.tile_pool(name="io", bufs=2) as io_pool,
            tc.tile_pool(name="rows", bufs=8) as rows_pool,
            tc.tile_pool(name="acc", bufs=2) as acc_pool,
        ):
            bias_sb = const_pool.tile([P, D], f32)
            nc.sync.dma_start(out=bias_sb[:], in_=bias_d[:])
            for s in range(2):
                for t in range(N_TILES):
                    rs = slice(t * P, (t + 1) * P)
                    idx_sb = io_pool.tile([P, K], i32, tag="idx")
                    val_sb = io_pool.tile([P, K], f32, tag="val")
                    nc.sync.dma_start(out=idx_sb[:], in_=idx_d[s][rs, :])
                    nc.sync.dma_start(out=val_sb[:], in_=val_d[s][rs, :])
                    acc = acc_pool.tile([P, D], f32, tag="acc")
                    for k in range(K):
                        rows = rows_pool.tile([P, D], f32, tag="rows")
                        nc.gpsimd.indirect_dma_start(
                            out=rows[:],
                            out_offset=None,
                            in_=w_d[:],
                            in_offset=bass.IndirectOffsetOnAxis(
                                ap=idx_sb[:, k : k + 1], axis=0
                            ),
                        )
                        scaled = rows_pool.tile([P, D], f32, tag="scaled")
                        nc.scalar.activation(
                            out=scaled[:],
                            in_=rows[:],
                            func=mybir.ActivationFunctionType.Copy,
                            scale=val_sb[:, k : k + 1],
                        )
                        if k == 0:
                            nc.vector.tensor_add(
                                out=acc[:], in0=scaled[:], in1=bias_sb[:]
                            )
                        else:
                            nc.vector.tensor_add(out=acc[:], in0=acc[:], in1=scaled[:])
                    nc.sync.dma_start(out=out_d[s][rs, :], in_=acc[:])

    nc.compile()
    return nc


def _get_nc():
    global _NC
    if _NC is None:
        _NC = _build()
    return _NC


def kernel(
    feature_indices_0,
    feature_values_0,
    feature_indices_1,
    feature_values_1,
    merged_weight,
    bias,
):
    global LAST_RESULT
    idx0 = np.ascontiguousarray(np.asarray(feature_indices_0, dtype=np.int32))
    idx1 = np.ascontiguousarray(np.asarray(feature_indices_1, dtype=np.int32))
    val0 = np.ascontiguousarray(np.asarray(feature_values_0, dtype=np.float32))
    val1 = np.ascontiguousarray(np.asarray(feature_values_1, dtype=np.float32))
    w = np.ascontiguousarray(np.asarray(merged_weight, dtype=np.float32))
    b = np.asarray(bias, dtype=np.float32)
    bias_rep = np.ascontiguousarray(np.broadcast_to(b[None, :], (P, D)))

    nc = _get_nc()
    in_maps = []
    for c in range(N_CORES):
        rs = slice(c * PER_CORE, (c + 1) * PER_CORE)
        in_maps.append(
            {
                "idx0": idx0[rs],
                "val0": val0[rs],
                "idx1": idx1[rs],
                "val1": val1[rs],
                "weight": w,
                "bias_rep": bias_rep,
            }
        )

    res = run_bass_kernel_spmd(
        nc, in_maps, core_ids=list(range(N_CORES)), trace=TRACE
    )
    LAST_RESULT = res
    out0 = np.concatenate([res.results[c]["out0"] for c in range(N_CORES)], axis=0)
    out1 = np.concatenate([res.results[c]["out1"] for c in range(N_CORES)], axis=0)
    return out0, out1



# revision 4
# speedup vs baseline: 1.3024x; 1.3024x over previous
"""Trainium2 Bass kernel for nn_ComposedFeatureTransformer (embedding lookup).

out_s[b, :] = bias + sum_k values_s[b, k] * merged_weight[indices_s[b, k], :]
for s in {0, 1}.

Strategy: data-parallel over the batch dim across 8 NeuronCores (512 rows
per core, both feature sets on every core). The weight table is converted
to fp16 on the host and row-padded to a 2304-byte stride (halves HBM
gather traffic; rel err ~1e-4, well under the 2e-2 gate). Rows are
fetched with the vectorized dma_gather ucode (1024 rows per instruction;
elem_step > elem_size skips the pad bytes). dma_gather takes int16
indices, so draws are split host-side into idx<32768 / idx>=32768 lists
gathered from two base addresses; each 128-draw group carries a
host-packed (dest-row, value) pair per lane from which DVE builds a
scatter-matrix stationary S[c,m] = val_c * (m == dest_c). The Tensor
engine then applies scale + scatter + K-accumulation as PSUM-accumulated
matmuls; bias is added during the PSUM->SBUF copy.
"""

import numpy as np

import concourse.bacc as bacc
import concourse.bass as bass
import concourse.mybir as mybir
import concourse.tile as tile
from concourse.bass import MemorySpace, ap_utils
from concourse.bass_utils import run_bass_kernel_spmd

N_CORES = 8
BATCH = 4096
PER_CORE = BATCH // N_CORES  # 512
K = 32
V = 45056
D = 1032
DP = 1152  # padded row stride in elements (2304 B, multiple of 256 B)
P = 128
N_TILES = PER_CORE // P  # 4
SPLIT = 32768  # int16 index ceiling for dma_gather
GRP_PER_INST = 8  # 1024 indices per gather instruction
# PSUM bank = 512 fp32; D=1032 split so no matmul crosses a bank boundary.
D_CHUNKS = [(0, 512), (512, 512), (1024, 8)]

TRACE = False  # set by test harness to collect an NTFF profile
LAST_RESULT = None  # BassKernelResults of the last run (for profiling)

_NC_CACHE = {}


def _dma_gather_raw(gp, out_ap, in_ap, idxs_ap, num_idxs, elem_size, elem_step):
    """bass.dma_gather minus the elem_size%256 assert (only real for
    transpose mode); elem_step > elem_size skips table row padding."""
    assert idxs_ap.dtype == mybir.dt.int16
    assert in_ap.space == MemorySpace.DRAM
    assert in_ap.dtype == out_ap.dtype
    assert idxs_ap.space == MemorySpace.SBUF
    assert out_ap.space == MemorySpace.SBUF
    assert ap_utils.ap_is_contiguous(in_ap.ap[1:])
    assert ap_utils.ap_is_contiguous(out_ap.ap[1:])
    assert ap_utils.ap_is_contiguous(idxs_ap.ap[1:])
    assert num_idxs % 128 == 0
    assert in_ap.ap[-1][1] == out_ap.ap[-1][1] == elem_size
    assert out_ap.ap[0][1] * out_ap.ap[1][1] == num_idxs
    assert in_ap.ap[0][0] == elem_step
    stride_bytes = elem_step * mybir.dt.size(in_ap.dtype)
    assert stride_bytes % 256 == 0
    stride_bytes_256 = stride_bytes // 256
    assert stride_bytes_256 < 256

    _in_ap = gp.lower_ap_dma(in_ap, for_custom_bir_dma=True)
    _idxs_ap = gp.lower_ap(idxs_ap)
    _out_ap = gp.lower_ap(out_ap)
    return gp.add_instruction(
        mybir.InstDMAGatherAnt(
            name=gp.bass.get_next_instruction_name(),
            ins=[*_in_ap, _idxs_ap, gp.lower_val_access(gp.to_reg(num_idxs))],
            outs=[_out_ap],
            transpose=False,
            num_idxs=num_idxs,
            elem_size=elem_size,
            stride_bytes_256=stride_bytes_256,
            gen_mode=0,
            single_packet=True,
            queue_num=0,
            sbuf_tokens_per_rank=0,
            sbuf_free_dim_per_rank=0,
            sbuf_free_dim_pad_per_rank=0,
            sbuf_byte_offset=0,
        )
    )


def _build(g_lo, g_hi):
    ga = g_lo + g_hi  # 128-draw groups per (set, tile)
    nc = bacc.Bacc("TRN2", debug=False, num_devices=N_CORES)
    f32 = mybir.dt.float32
    f16 = mybir.dt.float16
    i16 = mybir.dt.int16

    idx_d = [
        nc.dram_tensor(f"idx{s}", [N_TILES, P, ga * 8], i16, kind="ExternalInput")
        for s in range(2)
    ]
    m_d = [
        nc.dram_tensor(f"m{s}", [N_TILES, P, ga], f32, kind="ExternalInput")
        for s in range(2)
    ]
    v_d = [
        nc.dram_tensor(f"v{s}", [N_TILES, P, ga], f32, kind="ExternalInput")
        for s in range(2)
    ]
    w_d = nc.dram_tensor("wpad", [V, DP], f16, kind="ExternalInput")
    iota_d = nc.dram_tensor("iota", [P, P], f16, kind="ExternalInput")
    bias_d = nc.dram_tensor("bias_rep", [P, D], f32, kind="ExternalInput")
    out_d = [
        nc.dram_tensor(f"out{s}", [PER_CORE, D], f32, kind="ExternalOutput")
        for s in range(2)
    ]

    w_lo = w_d[0:SPLIT, 0:D]
    w_hi = w_d[SPLIT:V, 0:D]

    with tile.TileContext(nc) as tc:
        with (
            tc.tile_pool(name="const", bufs=1) as const_pool,
            tc.tile_pool(name="io", bufs=2) as io_pool,
            tc.tile_pool(name="rows", bufs=2) as rows_pool,
            tc.tile_pool(name="dgp", bufs=2) as dg_pool,
            tc.tile_pool(name="osb", bufs=2) as osb_pool,
            tc.psum_pool(name="accp", bufs=2) as acc_pool,
        ):
            bias_sb = const_pool.tile([P, D], f32)
            iota_sb = const_pool.tile([P, P], f16)
            nc.sync.dma_start(out=bias_sb[:], in_=bias_d[:])
            nc.sync.dma_start(out=iota_sb[:], in_=iota_d[:])
            for s in range(2):
                for t in range(N_TILES):
                    rs = slice(t * P, (t + 1) * P)
                    idx_sb = io_pool.tile([P, ga * 8], i16, tag="idx")
                    m_sb = io_pool.tile([P, ga], f32, tag="m")
                    v_sb = io_pool.tile([P, ga], f32, tag="v")
                    nc.sync.dma_start(out=idx_sb[:], in_=idx_d[s][t])
                    nc.sync.dma_start(out=m_sb[:], in_=m_d[s][t])
                    nc.sync.dma_start(out=v_sb[:], in_=v_d[s][t])

                    # Scatter stationaries S[c,m] = v_c * (m == dest_c).
                    dg = dg_pool.tile([P, ga, P], f16, tag="dg")
                    for g in range(ga):
                        nc.vector.tensor_scalar(
                            out=dg[:, g, :],
                            in0=iota_sb[:],
                            scalar1=m_sb[:, g : g + 1],
                            scalar2=v_sb[:, g : g + 1],
                            op0=mybir.AluOpType.is_equal,
                            op1=mybir.AluOpType.mult,
                        )

                    rows = rows_pool.tile([P, ga, D], f16, tag="rows")
                    for base, g0, gn in [(w_lo, 0, g_lo), (w_hi, g_lo, g_hi)]:
                        for c in range(0, gn, GRP_PER_INST):
                            gc = min(GRP_PER_INST, gn - c)
                            ni = gc * 128
                            _dma_gather_raw(
                                nc.gpsimd,
                                out_ap=rows[:, g0 + c : g0 + c + gc, :],
                                in_ap=base,
                                idxs_ap=idx_sb[
                                    :, (g0 + c) * 8 : (g0 + c + gc) * 8
                                ],
                                num_idxs=ni,
                                elem_size=D,
                                elem_step=DP,
                            )

                    acc = acc_pool.tile([P, D], f32, tag="acc")
                    for g in range(ga):
                        for c0, cn in D_CHUNKS:
                            nc.tensor.matmul(
                                out=acc[:, c0 : c0 + cn],
                                lhsT=dg[:, g, :],
                                rhs=rows[:, g, c0 : c0 + cn],
                                start=(g == 0),
                                stop=(g == ga - 1),
                            )

                    out_sb = osb_pool.tile([P, D], f32, tag="osb")
                    nc.vector.tensor_add(
                        out=out_sb[:], in0=acc[:, 0:D], in1=bias_sb[:]
                    )
                    nc.sync.dma_start(out=out_d[s][rs, :], in_=out_sb[:])

    nc.compile()
    return nc


def _get_nc(g_lo, g_hi):
    key = (g_lo, g_hi)
    if key not in _NC_CACHE:
        _NC_CACHE[key] = _build(g_lo, g_hi)
    return _NC_CACHE[key]


def _wrap16(flat_i16):
    """Wrap a chunk of n indices: element i -> (partition i%16, slot i//16),
    replicated into all 8 groups of 16 partitions (the gather ucode's tx/rx
    Q7 cores each read their own 16-channel copy)."""
    n = flat_i16.shape[0]
    w = flat_i16.reshape(n // 16, 16).T  # [16, n/16]
    return np.tile(w, (8, 1))  # [128, n/16]


def _pack_tile(idx, val, g_lo, g_hi):
    """Order one (set,tile)'s 128x32 draws lo-block then hi-block, pad each
    block to its global group count; return (idx16 [P, ga*8], m [P, ga],
    v [P, ga])."""
    b = np.broadcast_to(np.arange(P, dtype=np.int64)[:, None], idx.shape)
    fi = idx.ravel()
    fv = val.ravel()
    fb = b.ravel()
    lo = fi < SPLIT
    n_lo = int(lo.sum())
    n_hi = fi.size - n_lo
    ga = g_lo + g_hi

    di = np.zeros(ga * 128, np.int16)
    dv = np.zeros(ga * 128, np.float32)
    dm = np.zeros(ga * 128, np.float32)
    di[:n_lo] = fi[lo].astype(np.int16)
    dv[:n_lo] = fv[lo]
    dm[:n_lo] = fb[lo]
    h0 = g_lo * 128
    di[h0 : h0 + n_hi] = (fi[~lo] - SPLIT).astype(np.int16)
    dv[h0 : h0 + n_hi] = fv[~lo]
    dm[h0 : h0 + n_hi] = fb[~lo]

    # idx wrap is per gather instruction (chunks of GRP_PER_INST groups)
    cols = []
    for g0, gn in [(0, g_lo), (g_lo, g_hi)]:
        for c in range(0, gn, GRP_PER_INST):
            gc = min(GRP_PER_INST, gn - c)
            cols.append(_wrap16(di[(g0 + c) * 128 : (g0 + c + gc) * 128]))
    idx16 = np.concatenate(cols, axis=1)
    m = dm.reshape(ga, 128).T.copy()
    v = dv.reshape(ga, 128).T.copy()
    return idx16, m, v


def kernel(
    feature_indices_0,
    feature_values_0,
    feature_indices_1,
    feature_values_1,
    merged_weight,
    bias,
):
    global LAST_RESULT
    idx = [
        np.asarray(feature_indices_0, dtype=np.int64),
        np.asarray(feature_indices_1, dtype=np.int64),
    ]
    val = [
        np.asarray(feature_values_0, dtype=np.float32),
        np.asarray(feature_values_1, dtype=np.float32),
    ]
    w = np.asarray(merged_weight)
    wpad = np.zeros((V, DP), dtype=np.float16)
    wpad[:, :D] = w.astype(np.float16)
    b = np.asarray(bias, dtype=np.float32)
    bias_rep = np.ascontiguousarray(np.broadcast_to(b[None, :], (P, D)))
    iota = np.broadcast_to(
        np.arange(P, dtype=np.float16)[None, :], (P, P)
    ).copy()

    # global (max over cores/sets/tiles) group counts for lo/hi draws
    n_lo = np.array(
        [
            (idx[s][t * P + c * PER_CORE : (t + 1) * P + c * PER_CORE] < SPLIT)
            .sum()
            for c in range(N_CORES)
            for s in range(2)
            for t in range(N_TILES)
        ]
    )
    n_hi = P * K - n_lo
    g_lo = int(np.ceil(n_lo.max() / 128))
    g_hi = int(np.ceil(n_hi.max() / 128))
    ga = g_lo + g_hi

    nc = _get_nc(g_lo, g_hi)
    in_maps = []
    for c in range(N_CORES):
        im = {"wpad": wpad, "iota": iota, "bias_rep": bias_rep}
        for s in range(2):
            i16 = np.zeros((N_TILES, P, ga * 8), np.int16)
            mm = np.zeros((N_TILES, P, ga), np.float32)
            vv = np.zeros((N_TILES, P, ga), np.float32)
            for t in range(N_TILES):
                rs = slice(c * PER_CORE + t * P, c * PER_CORE + (t + 1) * P)
                i16[t], mm[t], vv[t] = _pack_tile(
                    idx[s][rs], val[s][rs], g_lo, g_hi
                )
            im[f"idx{s}"] = i16
            im[f"m{s}"] = mm
            im[f"v{s}"] = vv
        in_maps.append(im)

    res = run_bass_kernel_spmd(
        nc, in_maps, core_ids=list(range(N_CORES)), trace=TRACE
    )
    LAST_RESULT = res
    out0 = np.concatenate([res.results[c]["out0"] for c in range(N_CORES)], axis=0)
    out1 = np.concatenate([res.results[c]["out1"] for c in range(N_CORES)], axis=0)
    return out0, out1
